# revision 12
# baseline (speedup 1.0000x reference)
"""Trainium2 Bass kernel for nn_MultiHeadAttention_59227599012491.

Reference computation (per batch b):
    xf = x[b].reshape(S, 256)
    q  = softplus(xf @ Wq.T + bq);  k = softplus(xf @ Wk.T + bk)
    v  = xf @ Wv.T + bv
    out = ((q @ k.T) @ v) @ Wo.T + bo          (no softmax!)

No softmax -> associativity: out = q @ M + bo with
    G = k.T @ v   [256,256],   M = G @ Wo.T   [256,256]
so the S x S score matrix never exists. Sharding: B=4 batches x 2
query-halves -> 8 cores, no collectives; k/v/G/M are recomputed by both
cores of a pair (queries + output rows are split).

Per-core pipeline (all matmuls fp16, PE computes out = lhsT.T @ rhs):
    xbT  [256,4096]  x[b]^T, host-transposed, query half first
    kv loop (groups of 8 seq tiles): ps = x_tile @ [WkT|WvT] -> DVE
         +[bk|bv] -> fp16 kv tile; one batched ACT Softplus over the 8
         k-planes; GT[d,e] += v_tile^T k_tile accumulated in PSUM
         across all 32 tiles (G never leaves PSUM until the end).
    qT   [e,s] = Softplus(Wq x^T + bq): ACT applies the per-partition
         bias and softplus in one pass straight out of PSUM.
    M    = G @ WoT  (tiny)
    outT [do,s] = M^T q^T: transposed output so the bo bias is
         per-partition (DVE tensor_scalar_add) and the DRAM dump is
         contiguous 2KB runs per partition; host un-transposes.

DMA: inputs are packed host-side into 3 tensors (weights [256,1024],
x chunks [4,256,1024], biases [128,4] + bkv broadcast row) so the
whole input load is 7 DMA instructions (5 on sync, 2 on scalar) with
2KB descriptors; output is 4 DMA instructions of [128,1024] fp16.

The activation-table pass is steered to `softplus_and_others` so the
ACT engine loads its PWP table exactly once.
"""

import numpy as np

S = 4096
SQ = 2048  # query rows per core
D = 256
P = 128
IT = D // P  # 2 contraction tiles over d
NS = S // P  # 32 sequence tiles
GRP = 8  # kv tiles per softplus batch
NG = NS // GRP
NCH = 4  # x DMA chunks
CW = S // NCH
N_CORES = 8

_CACHE = {}


def _patched_act_tables(orig_fn):
    def patched(arch):
        tabs = orig_fn(arch)
        return {
            name: (s if name == "natural_log_exp_and_others" else set())
            for name, s in tabs.items()
        }

    return patched


def _build_nc():
    import concourse.bacc as bacc
    import concourse.mybir as mybir
    import concourse.tile as tile

    FP = mybir.dt.float32
    FR = mybir.dt.float16
    AF = mybir.ActivationFunctionType
    ADD = mybir.AluOpType.add

    nc = bacc.Bacc("TRN2", target_bir_lowering=False, debug=False, num_devices=1)

    xc_d = nc.declare_dram_parameter("xc", [NCH, D, CW], FR, isOutput=False)
    wpack_d = nc.declare_dram_parameter("wpack", [D, 4 * D], FR, isOutput=False)
    bias_d = nc.declare_dram_parameter("biasp", [P, 4], FP, isOutput=False)
    bkv_d = nc.declare_dram_parameter("bkv", [1, 2 * D], FP, isOutput=False)
    outp_d = nc.declare_dram_parameter("outp", [P, 2 * SQ], FR, isOutput=True)

    def mm(psum, lhsT, rhs, start, stop):
        nc.tensor.matmul(psum, lhsT, rhs, start=start, stop=stop)

    with tile.TileContext(nc) as tc:
        with (
            tc.tile_pool(name="w", bufs=1) as wpool,
            tc.tile_pool(name="big", bufs=1) as big,
            tc.tile_pool(name="kvg", bufs=2) as kvg,
            tc.tile_pool(name="tmp", bufs=2) as tpool,
            tc.tile_pool(name="psKV", bufs=3, space="PSUM") as psKV,
            tc.tile_pool(name="psG", bufs=1, space="PSUM") as psG,
            tc.tile_pool(name="psQ", bufs=1, space="PSUM") as psQ,
        ):
            w_sb = wpool.tile([P, IT, 4 * D], FR, tag="w")
            xbT_sb = big.tile([P, IT, S], FR, tag="xbT")
            bias_sb = wpool.tile([P, 4], FP, tag="bias")
            bkv_bc = wpool.tile([P, 2 * D], FP, tag="bkv")
            qT_sb = big.tile([P, IT, SQ], FR, tag="qT")
            GT_sb = wpool.tile([P, IT, D], FR, tag="GT")
            M_sb = wpool.tile([P, IT, D], FR, tag="M")
            outT_sb = big.tile([P, IT, SQ], FR, tag="outT")

            # --- input DMAs: weights + x chunks on sync, biases on scalar ---
            nc.sync.dma_start(
                w_sb[:, :, :],
                wpack_d.ap()[:, :].rearrange("(i p) w -> p i w", p=P),
            )
            for c in range(NCH):
                nc.sync.dma_start(
                    xbT_sb[:, :, c * CW : (c + 1) * CW],
                    xc_d.ap()[c, :, :].rearrange("(i p) w -> p i w", p=P),
                )
            nc.scalar.dma_start(bkv_bc[:, :], bkv_d.ap()[0:1, :].broadcast_to([P, 2 * D]))
            nc.scalar.dma_start(bias_sb[:, :], bias_d.ap()[:, :])

            wkv = w_sb[:, :, D : 3 * D]

            # --- kv + GT: groups of 8 seq tiles; GT accumulates in PSUM ---
            GTps = []
            for dt in range(IT):
                gt = psG.tile([P, D], FP, tag=f"psG{dt}", name=f"GTps{dt}")
                GTps.append(gt)
            for g in range(NG):
                kvt = kvg.tile([P, GRP, 2, D], FR, tag="kv")
                for i in range(GRP):
                    t = g * GRP + i
                    ts = slice(t * P, (t + 1) * P)
                    ps = psKV.tile([P, 2 * D], FP, tag="psKV")
                    for it in range(IT):
                        mm(ps[:, :], xbT_sb[:, it, ts], wkv[:, it, :], it == 0, it == IT - 1)
                    nc.vector.tensor_tensor(
                        kvt[:, i, :, :],
                        ps[:, :].rearrange("p (j d) -> p j d", j=2),
                        bkv_bc[:, :].rearrange("p (j d) -> p j d", j=2),
                        op=ADD,
                    )
                # batched softplus over the 8 k-planes: Exp then Ln(1+t)
                tmpk = tpool.tile([P, GRP, D], FP, tag="tmpk")
                nc.scalar.activation(tmpk[:, :, :], kvt[:, :, 0, :], AF.Exp)
                nc.scalar.activation(kvt[:, :, 0, :], tmpk[:, :, :], AF.Ln, bias=1.0)
                for i in range(GRP):
                    for dt in range(IT):
                        mm(
                            GTps[dt],
                            kvt[:, i, 1, dt * P : (dt + 1) * P],
                            kvt[:, i, 0, :],
                            g == 0 and i == 0,
                            g == NG - 1 and i == GRP - 1,
                        )

            # --- qT = softplus(Wq x^T + bq), [e, s], bias+softplus in one ACT ---
            for dt in range(IT):
                for h in range(SQ // 1024):
                    ps = psQ.tile([P, 2, 512], FP, tag="psQ")
                    for c in range(2):
                        ss = slice((2 * h + c) * 512, (2 * h + c + 1) * 512)
                        for it in range(IT):
                            mm(
                                ps[:, c, :],
                                w_sb[:, it, dt * P : (dt + 1) * P],
                                xbT_sb[:, it, ss],
                                it == 0,
                                it == IT - 1,
                            )
                    tmpq = tpool.tile([P, 1024], FP, tag="tmpq")
                    nc.scalar.activation(
                        tmpq[:, :],
                        ps[:, :, :].rearrange("p a b -> p (a b)"),
                        AF.Exp,
                        bias=bias_sb[:, dt : dt + 1],
                    )
                    nc.scalar.activation(
                        qT_sb[:, dt, h * 1024 : (h + 1) * 1024], tmpq[:, :], AF.Ln, bias=1.0
                    )

            # --- GT -> SBUF, M = GT^T @ WoT ---
            for dt in range(IT):
                nc.vector.tensor_copy(GT_sb[:, dt, :], GTps[dt][:, :])
            for et in range(IT):
                ps = psQ.tile([P, 2, 512], FP, tag="psQ")
                for dt in range(IT):
                    mm(
                        ps[:, 0, 0:D],
                        GT_sb[:, dt, et * P : (et + 1) * P],
                        w_sb[:, dt, 3 * D : 4 * D],
                        dt == 0,
                        dt == IT - 1,
                    )
                nc.vector.tensor_copy(M_sb[:, et, :], ps[:, 0, 0:D])

            # --- outT[do, s] = M^T q^T + bo (per-partition bias) ---
            for dot in range(IT):
                for h in range(SQ // 1024):
                    ps = psQ.tile([P, 2, 512], FP, tag="psQ")
                    for c in range(2):
                        ss = slice((2 * h + c) * 512, (2 * h + c + 1) * 512)
                        for et in range(IT):
                            mm(
                                ps[:, c, :],
                                M_sb[:, et, dot * P : (dot + 1) * P],
                                qT_sb[:, et, ss],
                                et == 0,
                                et == IT - 1,
                            )
                    nc.vector.tensor_scalar_add(
                        outT_sb[:, dot, h * 1024 : (h + 1) * 1024],
                        ps[:, :, :].rearrange("p a b -> p (a b)"),
                        bias_sb[:, 2 + dot : 3 + dot],
                    )
                    nc.sync.dma_start(
                        outp_d.ap()[:, dot * SQ + h * 1024 : dot * SQ + (h + 1) * 1024],
                        outT_sb[:, dot, h * 1024 : (h + 1) * 1024],
                    )

    import concourse.hw_specs as hw_specs

    orig = bacc.get_activation_tables
    bacc.get_activation_tables = _patched_act_tables(hw_specs.get_activation_tables)
    try:
        nc.compile()
    finally:
        bacc.get_activation_tables = orig
    return nc


def _get_nc():
    nc = _CACHE.get("nc")
    if nc is None:
        nc = _build_nc()
        _CACHE["nc"] = nc
    return nc


def make_in_maps(x, Wq, bq, Wk, bk, Wv, bv, Wo, bo):
    B = x.shape[0]
    f16 = np.float16
    xf = np.asarray(x, dtype=np.float32).reshape(B, S, D)
    xfT = np.ascontiguousarray(xf.transpose(0, 2, 1).astype(f16))  # [B, 256, 4096]
    wpack = np.hstack(
        [
            np.asarray(Wq, f16).T,
            np.asarray(Wk, f16).T,
            np.asarray(Wv, f16).T,
            np.asarray(Wo, f16).T,
        ]
    )
    biasp = np.stack(
        [
            np.asarray(bq, np.float32)[0:P],
            np.asarray(bq, np.float32)[P:D],
            np.asarray(bo, np.float32)[0:P],
            np.asarray(bo, np.float32)[P:D],
        ],
        axis=1,
    )
    shared = {
        "wpack": np.ascontiguousarray(wpack),
        "biasp": np.ascontiguousarray(biasp),
        "bkv": np.concatenate(
            [np.asarray(bk, np.float32), np.asarray(bv, np.float32)]
        ).reshape(1, 2 * D),
    }
    in_maps = []
    for c in range(N_CORES):
        b, h = divmod(c, 2)
        xT = xfT[b]
        if h == 1:
            xT = np.concatenate([xT[:, SQ:], xT[:, :SQ]], axis=1)
        xc = np.ascontiguousarray(xT.reshape(D, NCH, CW).transpose(1, 0, 2))
        in_maps.append({"xc": xc, **shared})
    return in_maps


def assemble_out(results, x_shape):
    B, S_, H, W = x_shape
    out = np.empty((B, S_, D), np.float32)
    for c in range(N_CORES):
        b, h = divmod(c, 2)
        outp = results[c]["outp"]  # [128, 2*SQ] fp16: [p, dot*SQ + s]
        v = outp.reshape(P, IT, SQ).astype(np.float32)
        out[b, h * SQ : (h + 1) * SQ] = v.transpose(2, 1, 0).reshape(SQ, D)
    return out.reshape(B, S_, H, W)


def kernel(x, Wq, bq, Wk, bk, Wv, bv, Wo, bo, _trace=False):
    from concourse.bass_utils import run_bass_kernel_spmd

    nc = _get_nc()
    in_maps = make_in_maps(x, Wq, bq, Wk, bk, Wv, bv, Wo, bo)
    res = run_bass_kernel_spmd(nc, in_maps, list(range(N_CORES)), trace=_trace)
    out = assemble_out(res.results, x.shape)
    if _trace:
        _CACHE["last_result"] = res
    return out


# revision 15
# speedup vs baseline: 1.0142x; 1.0142x over previous
"""Trainium2 Bass kernel for nn_MultiHeadAttention_59227599012491.

Reference computation (per batch b):
    xf = x[b].reshape(S, 256)
    q  = softplus(xf @ Wq.T + bq);  k = softplus(xf @ Wk.T + bk)
    v  = xf @ Wv.T + bv
    out = ((q @ k.T) @ v) @ Wo.T + bo          (no softmax!)

No softmax -> associativity: out = q @ M + bo with
    G = k.T @ v   [256,256],   M = G @ Wo.T   [256,256]
so the S x S score matrix never exists. Sharding: B=4 batches x 2
query-halves -> 8 cores, no collectives (an NRT AllReduce of M was
measured at ~17 us fixed latency -- more than the whole dedup saves, so
k/v/G/M are recomputed by both cores of a pair; queries + output rows
are split).

Per-core pipeline (all matmuls fp16, PE computes out = lhsT.T @ rhs):
    kv loop (4 groups of 8 seq tiles): ps = x_tile @ [WkT|WvT]; DVE
        adds bk to the k plane, GpSimd adds bv to the v plane (psum ->
        fp16); batched ACT Exp+Ln softplus over each group's k planes;
        GT[d,e] += v_tile^T k_tile accumulated in PSUM across all 32
        tiles. One qT chunk is interleaved after each group so the ACT
        engine's softplus backlog hides under PE work.
    qT [e,s] = softplus(Wq x^T + bq): per-partition bias fused into the
        ACT Exp pass straight out of PSUM.
    M = G @ WoT (tiny), then outT [do,s] = M^T q^T + bo: transposed
        output so bo is per-partition (DVE tensor_scalar_add) and the
        fp16 DRAM dump is contiguous 2 KB runs per partition; the host
        un-transposes and casts back to fp32.

DMA: every input DMA moves 2 KB descriptors (host-packed layouts); each
DMA instruction occupies one HW queue (~60 GB/s at 2 KB descriptors),
so the load is split into pieces across three issuing engines (sync +
scalar HWDGE, gpsimd SWDGE) for queue parallelism, with the
first-needed pieces (Wkv, x cols 0:1024) split by partition halves to
land earliest. Output: 4 chunks of [128,1024] fp16, each written as two
[64,1024] pieces on alternating queues so the final chunk drains fast.

The activation-table pass is steered to `natural_log_exp_and_others`
(the only set holding Exp AND Ln) so the ACT engine loads its PWP
table exactly once.
"""

import numpy as np

S = 4096
SQ = 2048  # query rows per core
D = 256
P = 128
IT = D // P  # 2 contraction tiles over d
NS = S // P  # 32 sequence tiles
GRP = 8  # kv tiles per softplus batch
NG = NS // GRP
N_CORES = 8

_CACHE = {}


def _patched_act_tables(orig_fn):
    def patched(arch):
        tabs = orig_fn(arch)
        return {
            name: (s if name == "natural_log_exp_and_others" else set())
            for name, s in tabs.items()
        }

    return patched


def _build_nc():
    import concourse.bacc as bacc
    import concourse.mybir as mybir
    import concourse.tile as tile

    FP = mybir.dt.float32
    FR = mybir.dt.float16
    AF = mybir.ActivationFunctionType
    ADD = mybir.AluOpType.add

    nc = bacc.Bacc("TRN2", target_bir_lowering=False, debug=False, num_devices=1)

    # x pieces: [8, 128, 1024], piece it*4+cc = x^T[it-block, cc*1024:...]
    xp_d = nc.declare_dram_parameter("xp", [2 * 4, P, 1024], FR, isOutput=False)
    wkv_d = nc.declare_dram_parameter("wkvp", [P, 2 * 512], FR, isOutput=False)
    wqo_d = nc.declare_dram_parameter("wqop", [P, 2 * 512], FR, isOutput=False)
    bias_d = nc.declare_dram_parameter("biasp", [P, 4], FP, isOutput=False)
    bkv_d = nc.declare_dram_parameter("bkv", [1, 2 * D], FP, isOutput=False)
    outp_d = nc.declare_dram_parameter("outp", [P, 2 * SQ], FR, isOutput=True)

    def mm(psum, lhsT, rhs, start, stop):
        nc.tensor.matmul(psum, lhsT, rhs, start=start, stop=stop)

    with tile.TileContext(nc) as tc:
        with (
            tc.tile_pool(name="w", bufs=1) as wpool,
            tc.tile_pool(name="big", bufs=1) as big,
            tc.tile_pool(name="kvg", bufs=2) as kvg,
            tc.tile_pool(name="tmp", bufs=2) as tpool,
            tc.tile_pool(name="psKV", bufs=2, space="PSUM") as psKV,
            tc.tile_pool(name="psG", bufs=1, space="PSUM") as psG,
            tc.tile_pool(name="psQ", bufs=2, space="PSUM") as psQ,
        ):
            # SBUF weight layout: cols [wkv 512 | wq 256 | wo 256] per it
            w_sb = wpool.tile([P, IT, 1024], FR, tag="w")
            xbT_sb = big.tile([P, IT, S], FR, tag="xbT")
            bias_sb = wpool.tile([P, 4], FP, tag="bias")
            bkv_bc = wpool.tile([P, 2 * D], FP, tag="bkv")
            qT_sb = big.tile([P, IT, SQ], FR, tag="qT")
            GT_sb = wpool.tile([P, IT, D], FR, tag="GT")
            M_sb = wpool.tile([P, IT, D], FR, tag="M")
            outT_sb = big.tile([P, IT, SQ], FR, tag="outT")

            # --- input DMAs: one HW queue per instruction; first-needed
            # pieces split by partition halves across three engines ---
            nc.sync.dma_start(
                w_sb[0:64, :, 0:512],
                wkv_d.ap()[0:64, :].rearrange("p (i w) -> p i w", i=IT),
            )
            nc.scalar.dma_start(
                w_sb[64:P, :, 0:512],
                wkv_d.ap()[64:P, :].rearrange("p (i w) -> p i w", i=IT),
            )
            # x cols 0:1024 (pieces 0 and 4), split into partition halves
            for half in range(2):
                hp = slice(half * 64, (half + 1) * 64)
                eng = nc.sync if half == 0 else nc.scalar
                eng.dma_start(xbT_sb[hp, 0, 0:1024], xp_d.ap()[0, hp, :])
                eng.dma_start(xbT_sb[hp, 1, 0:1024], xp_d.ap()[4, hp, :])
            nc.gpsimd.dma_start(
                bkv_bc[:, :], bkv_d.ap()[0:1, :].broadcast_to([P, 2 * D])
            )
            nc.gpsimd.dma_start(bias_sb[:, :], bias_d.ap()[:, :])
            # remaining x pieces + wqo spread over the three engines
            rest = [(it, cc) for cc in range(1, 4) for it in range(IT)]
            engs = [nc.sync, nc.scalar, nc.gpsimd]
            for n, (it, cc) in enumerate(rest):
                engs[n % 3].dma_start(
                    xbT_sb[:, it, cc * 1024 : (cc + 1) * 1024],
                    xp_d.ap()[it * 4 + cc, :, :],
                )
            nc.gpsimd.dma_start(
                w_sb[:, :, 512:1024],
                wqo_d.ap()[:, :].rearrange("p (i w) -> p i w", i=IT),
            )

            wkv = w_sb[:, :, 0:512]

            # --- kv + GT loop; one qT chunk interleaved per group ---
            GTps = []
            for dt in range(IT):
                gt = psG.tile([P, D], FP, tag=f"psG{dt}", name=f"GTps{dt}")
                GTps.append(gt)
            for g in range(NG):
                kvt = kvg.tile([P, GRP, 2, D], FR, tag="kv")
                for i in range(GRP):
                    t = g * GRP + i
                    ts = slice(t * P, (t + 1) * P)
                    ps = psKV.tile([P, 2 * D], FP, tag="psKV")
                    for it in range(IT):
                        mm(ps[:, :], xbT_sb[:, it, ts], wkv[:, it, :], it == 0, it == IT - 1)
                    nc.vector.tensor_tensor(
                        kvt[:, i, :, :],
                        ps[:, :].rearrange("p (j d) -> p j d", j=2),
                        bkv_bc[:, :].rearrange("p (j d) -> p j d", j=2),
                        op=ADD,
                    )
                # batched softplus over the group's k planes: Exp then Ln(1+t)
                tmpk = tpool.tile([P, GRP, D], FP, tag="tmpk")
                nc.scalar.activation(tmpk[:, :, :], kvt[:, :, 0, :], AF.Exp)
                nc.scalar.activation(kvt[:, :, 0, :], tmpk[:, :, :], AF.Ln, bias=1.0)
                for i in range(GRP):
                    for dt in range(IT):
                        mm(
                            GTps[dt],
                            kvt[:, i, 1, dt * P : (dt + 1) * P],
                            kvt[:, i, 0, :],
                            g == 0 and i == 0,
                            g == NG - 1 and i == GRP - 1,
                        )
                # one qT chunk (dt, h) per group: softplus(Wq x^T + bq)
                dt, h = divmod(g, 2)
                ps = psQ.tile([P, 2, 512], FP, tag="psQ")
                for c in range(2):
                    ss = slice((2 * h + c) * 512, (2 * h + c + 1) * 512)
                    for it in range(IT):
                        mm(
                            ps[:, c, :],
                            w_sb[:, it, 512 + dt * P : 512 + (dt + 1) * P],
                            xbT_sb[:, it, ss],
                            it == 0,
                            it == IT - 1,
                        )
                tmpq = tpool.tile([P, 1024], FP, tag="tmpq")
                nc.scalar.activation(
                    tmpq[:, :],
                    ps[:, :, :].rearrange("p a b -> p (a b)"),
                    AF.Exp,
                    bias=bias_sb[:, dt : dt + 1],
                )
                nc.scalar.activation(
                    qT_sb[:, dt, h * 1024 : (h + 1) * 1024], tmpq[:, :], AF.Ln, bias=1.0
                )

            # --- GT -> SBUF, M = GT^T @ WoT ---
            for dt in range(IT):
                nc.vector.tensor_copy(GT_sb[:, dt, :], GTps[dt][:, :])
            for et in range(IT):
                ps = psQ.tile([P, 2, 512], FP, tag="psQ")
                for dt in range(IT):
                    mm(
                        ps[:, 0, 0:D],
                        GT_sb[:, dt, et * P : (et + 1) * P],
                        w_sb[:, dt, 768:1024],
                        dt == 0,
                        dt == IT - 1,
                    )
                nc.vector.tensor_copy(M_sb[:, et, :], ps[:, 0, 0:D])

            # --- outT[do, s] = M^T q^T + bo (per-partition bias) ---
            for dot in range(IT):
                for h in range(SQ // 1024):
                    ps = psQ.tile([P, 2, 512], FP, tag="psQ")
                    for c in range(2):
                        ss = slice((2 * h + c) * 512, (2 * h + c + 1) * 512)
                        for et in range(IT):
                            mm(
                                ps[:, c, :],
                                M_sb[:, et, dot * P : (dot + 1) * P],
                                qT_sb[:, et, ss],
                                et == 0,
                                et == IT - 1,
                            )
                    cs = slice(h * 1024, (h + 1) * 1024)
                    nc.vector.tensor_scalar_add(
                        outT_sb[:, dot, cs],
                        ps[:, :, :].rearrange("p a b -> p (a b)"),
                        bias_sb[:, 2 + dot : 3 + dot],
                    )
                    off = dot * SQ
                    for half in range(2):
                        hp = slice(half * 64, (half + 1) * 64)
                        eng = nc.sync if half == 0 else nc.scalar
                        eng.dma_start(
                            outp_d.ap()[hp, off + h * 1024 : off + (h + 1) * 1024],
                            outT_sb[hp, dot, cs],
                        )

    import concourse.hw_specs as hw_specs

    orig = bacc.get_activation_tables
    bacc.get_activation_tables = _patched_act_tables(hw_specs.get_activation_tables)
    try:
        nc.compile()
    finally:
        bacc.get_activation_tables = orig
    return nc


def _get_nc():
    nc = _CACHE.get("nc")
    if nc is None:
        nc = _build_nc()
        _CACHE["nc"] = nc
    return nc


def make_in_maps(x, Wq, bq, Wk, bk, Wv, bv, Wo, bo):
    B = x.shape[0]
    f16 = np.float16
    xf = np.asarray(x, dtype=np.float32).reshape(B, S, D)
    xfT = np.ascontiguousarray(xf.transpose(0, 2, 1).astype(f16))  # [B, 256, 4096]

    def pack_it(wT):  # [256, C] -> [128, 2*C] with it-blocks side by side
        C = wT.shape[1]
        return np.ascontiguousarray(
            wT.reshape(IT, P, C).transpose(1, 0, 2).reshape(P, IT * C)
        )

    wkvp = pack_it(np.hstack([np.asarray(Wk, f16).T, np.asarray(Wv, f16).T]))
    wqop = pack_it(np.hstack([np.asarray(Wq, f16).T, np.asarray(Wo, f16).T]))
    biasp = np.stack(
        [
            np.asarray(bq, np.float32)[0:P],
            np.asarray(bq, np.float32)[P:D],
            np.asarray(bo, np.float32)[0:P],
            np.asarray(bo, np.float32)[P:D],
        ],
        axis=1,
    )
    shared = {
        "wkvp": wkvp,
        "wqop": wqop,
        "biasp": np.ascontiguousarray(biasp),
        "bkv": np.concatenate(
            [np.asarray(bk, np.float32), np.asarray(bv, np.float32)]
        ).reshape(1, 2 * D),
    }
    in_maps = []
    for c in range(N_CORES):
        b, h = divmod(c, 2)
        xT = xfT[b]
        if h == 1:
            xT = np.concatenate([xT[:, SQ:], xT[:, :SQ]], axis=1)
        # pieces [it*4+cc] = [128, 1024]
        xpc = np.ascontiguousarray(
            xT.reshape(IT, P, 4, 1024).transpose(0, 2, 1, 3).reshape(8, P, 1024)
        )
        in_maps.append({"xp": xpc, **shared})
    return in_maps


def assemble_out(results, x_shape):
    B, S_, H, W = x_shape
    out = np.empty((B, S_, D), np.float32)
    for c in range(N_CORES):
        b, h = divmod(c, 2)
        outp = results[c]["outp"]  # [128, 2*SQ] fp16: [p, dot*SQ + s]
        v = outp.reshape(P, IT, SQ).astype(np.float32)
        out[b, h * SQ : (h + 1) * SQ] = v.transpose(2, 1, 0).reshape(SQ, D)
    return out.reshape(B, S_, H, W)


def kernel(x, Wq, bq, Wk, bk, Wv, bv, Wo, bo, _trace=False):
    from concourse.bass_utils import run_bass_kernel_spmd

    nc = _get_nc()
    in_maps = make_in_maps(x, Wq, bq, Wk, bk, Wv, bv, Wo, bo)
    res = run_bass_kernel_spmd(nc, in_maps, list(range(N_CORES)), trace=_trace)
    out = assemble_out(res.results, x.shape)
    if _trace:
        _CACHE["last_result"] = res
    return out


# revision 16
# speedup vs baseline: 1.2764x; 1.2584x over previous
"""Trainium2 Bass kernel for nn_MultiHeadAttention_59227599012491.

Reference computation (per batch b):
    xf = x[b].reshape(S, 256)
    q  = softplus(xf @ Wq.T + bq);  k = softplus(xf @ Wk.T + bk)
    v  = xf @ Wv.T + bv
    out = ((q @ k.T) @ v) @ Wo.T + bo          (no softmax!)

No softmax -> associativity: out = q @ M + bo with
    G = k.T @ v   [256,256],   M = G @ Wo.T   [256,256]
so the S x S score matrix never exists. Sharding: B=4 batches x 2
query-halves -> 8 cores, no collectives (an NRT AllReduce of M was
measured at ~17 us fixed latency -- more than the whole dedup saves, so
k/v/G/M are recomputed by both cores of a pair; queries + output rows
are split).

Per-core pipeline (all matmuls fp16, PE computes out = lhsT.T @ rhs):
    kv loop (4 groups of 8 seq tiles): ps = x_tile @ [WkT|WvT]; DVE
        adds bk to the k plane, GpSimd adds bv to the v plane (psum ->
        fp16); batched ACT Exp+Ln softplus over each group's k planes;
        GT[d,e] += v_tile^T k_tile accumulated in PSUM across all 32
        tiles. One qT chunk is interleaved after each group so the ACT
        engine's softplus backlog hides under PE work.
    qT [e,s] = softplus(Wq x^T + bq): per-partition bias fused into the
        ACT Exp pass straight out of PSUM.
    M = G @ WoT (tiny), then outT [do,s] = M^T q^T + bo: transposed
        output so bo is per-partition (DVE tensor_scalar_add) and the
        fp16 DRAM dump is contiguous 2 KB runs per partition; the host
        un-transposes and casts back to fp32.

DMA: every input DMA moves 2 KB descriptors (host-packed layouts); each
DMA instruction occupies one HW queue (~60 GB/s at 2 KB descriptors),
so the load is split into pieces across three issuing engines (sync +
scalar HWDGE, gpsimd SWDGE) for queue parallelism, with the
first-needed pieces (Wkv, x cols 0:1024) split by partition halves to
land earliest. Output: 4 chunks of [128,1024] fp16, each written as two
[64,1024] pieces on alternating queues so the final chunk drains fast.

The activation-table pass is steered to `natural_log_exp_and_others`
(the only set holding Exp AND Ln) so the ACT engine loads its PWP
table exactly once.
"""

import numpy as np

S = 4096
SQ = 2048  # query rows per core
D = 256
P = 128
IT = D // P  # 2 contraction tiles over d
NS = S // P  # 32 sequence tiles
GRP = 8  # kv tiles per softplus batch
NG = NS // GRP
N_CORES = 8

_CACHE = {}


def _patched_act_tables(orig_fn):
    def patched(arch):
        tabs = orig_fn(arch)
        return {
            name: (s if name == "natural_log_exp_and_others" else set())
            for name, s in tabs.items()
        }

    return patched


def _build_nc():
    import concourse.bacc as bacc
    import concourse.mybir as mybir
    import concourse.tile as tile

    FP = mybir.dt.float32
    FR = mybir.dt.float16
    AF = mybir.ActivationFunctionType
    ADD = mybir.AluOpType.add

    nc = bacc.Bacc("TRN2", target_bir_lowering=False, debug=False, num_devices=1)

    # x pieces: [8, 128, 1024], piece it*4+cc = x^T[it-block, cc*1024:...]
    xp_d = nc.declare_dram_parameter("xp", [2 * 4, P, 1024], FR, isOutput=False)
    wkv_d = nc.declare_dram_parameter("wkvp", [P, 2 * 512], FR, isOutput=False)
    wqo_d = nc.declare_dram_parameter("wqop", [P, 2 * 512], FR, isOutput=False)
    bias_d = nc.declare_dram_parameter("biasp", [P, 4], FP, isOutput=False)
    bkv_d = nc.declare_dram_parameter("bkv", [1, 2 * D], FP, isOutput=False)
    outp_d = nc.declare_dram_parameter("outp", [P, 2 * SQ], FR, isOutput=True)

    def mm(psum, lhsT, rhs, start, stop):
        nc.tensor.matmul(psum, lhsT, rhs, start=start, stop=stop)

    with tile.TileContext(nc) as tc:
        with (
            tc.tile_pool(name="w", bufs=1) as wpool,
            tc.tile_pool(name="big", bufs=1) as big,
            tc.tile_pool(name="tmp", bufs=4) as tpool,
            tc.tile_pool(name="psKV", bufs=3, space="PSUM") as psKV,
            tc.tile_pool(name="psG", bufs=1, space="PSUM") as psG,
            tc.tile_pool(name="psQ", bufs=3, space="PSUM") as psQ,
        ):
            # SBUF weight layout: cols [wkv 512 | wq 256 | wo 256] per it
            w_sb = wpool.tile([P, IT, 1024], FR, tag="w")
            xbT_sb = big.tile([P, IT, S], FR, tag="xbT")
            bias_sb = wpool.tile([P, 4], FP, tag="bias")
            bkv_bc = wpool.tile([P, 2 * D], FP, tag="bkv")
            kv_sb = big.tile([P, 2, NS, D], FR, tag="kv")
            qT_sb = big.tile([P, IT, SQ], FR, tag="qT")
            GT_sb = wpool.tile([P, IT, D], FR, tag="GT")
            M_sb = wpool.tile([P, IT, D], FR, tag="M")
            outT_sb = big.tile([P, IT, SQ], FR, tag="outT")

            # --- input DMAs: one HW queue per instruction, spread over
            # sync/scalar (HWDGE) + gpsimd (SWDGE); first-needed first ---
            nc.sync.dma_start(
                w_sb[0:64, :, 0:512],
                wkv_d.ap()[0:64, :].rearrange("p (i w) -> p i w", i=IT),
            )
            nc.scalar.dma_start(
                w_sb[64:P, :, 0:512],
                wkv_d.ap()[64:P, :].rearrange("p (i w) -> p i w", i=IT),
            )
            nc.gpsimd.dma_start(
                bkv_bc[:, :], bkv_d.ap()[0:1, :].broadcast_to([P, 2 * D])
            )
            nc.gpsimd.dma_start(bias_sb[:, :], bias_d.ap()[:, :])

            def xpiece(eng, it, cc):
                eng.dma_start(
                    xbT_sb[:, it, cc * 1024 : (cc + 1) * 1024],
                    xp_d.ap()[it * 4 + cc, :, :],
                )

            xpiece(nc.sync, 0, 0)
            xpiece(nc.scalar, 1, 0)
            xpiece(nc.sync, 0, 1)
            xpiece(nc.gpsimd, 1, 1)
            xpiece(nc.scalar, 0, 2)
            xpiece(nc.sync, 1, 2)
            xpiece(nc.scalar, 0, 3)
            xpiece(nc.sync, 1, 3)
            nc.gpsimd.dma_start(
                w_sb[:, :, 512:1024],
                wqo_d.ap()[:, :].rearrange("p (i w) -> p i w", i=IT),
            )

            wkv = w_sb[:, :, 0:512]

            # --- kv = x [WkT | WvT] + [bk | bv]; softplus k in 4-tile runs ---
            for t in range(NS):
                ts = slice(t * P, (t + 1) * P)
                ps = psKV.tile([P, 2 * D], FP, tag="psKV")
                for it in range(IT):
                    mm(ps[:, :], xbT_sb[:, it, ts], wkv[:, it, :], it == 0, it == IT - 1)
                nc.vector.tensor_tensor(
                    kv_sb[:, :, t, :],
                    ps[:, :].rearrange("p (j d) -> p j d", j=2),
                    bkv_bc[:, :].rearrange("p (j d) -> p j d", j=2),
                    op=ADD,
                )
                if t % 4 == 3:
                    tt = slice(t - 3, t + 1)
                    tmpk = tpool.tile([P, 4, D], FP, tag="tmpk")
                    nc.scalar.activation(tmpk[:, :, :], kv_sb[:, 0, tt, :], AF.Exp)
                    nc.scalar.activation(kv_sb[:, 0, tt, :], tmpk[:, :, :], AF.Ln, bias=1.0)

            # --- qT = softplus(Wq x^T + bq), [e, s]; bias fused in Exp ---
            for dt in range(IT):
                for blk in range(SQ // 512):
                    ss = slice(blk * 512, (blk + 1) * 512)
                    ps = psQ.tile([P, 512], FP, tag="psQ")
                    for it in range(IT):
                        mm(
                            ps[:, :],
                            w_sb[:, it, 512 + dt * P : 512 + (dt + 1) * P],
                            xbT_sb[:, it, ss],
                            it == 0,
                            it == IT - 1,
                        )
                    tmpq = tpool.tile([P, 512], FP, tag="tmpq")
                    nc.scalar.activation(
                        tmpq[:, :], ps[:, :], AF.Exp, bias=bias_sb[:, dt : dt + 1]
                    )
                    nc.scalar.activation(qT_sb[:, dt, ss], tmpq[:, :], AF.Ln, bias=1.0)

            # --- GT[d, e] = sum_s v[s, d] k[s, e] (pure PE streaming) ---
            GTps = []
            for dt in range(IT):
                gt = psG.tile([P, D], FP, tag=f"psG{dt}", name=f"GTps{dt}")
                GTps.append(gt)
            for dt in range(IT):
                vs = slice(dt * P, (dt + 1) * P)
                for t in range(NS):
                    mm(GTps[dt], kv_sb[:, 1, t, vs], kv_sb[:, 0, t, :], t == 0, t == NS - 1)
                nc.vector.tensor_copy(GT_sb[:, dt, :], GTps[dt][:, :])

            # --- M = GT^T @ WoT ---
            for et in range(IT):
                ps = psQ.tile([P, 512], FP, tag="psQ")
                for dt in range(IT):
                    mm(
                        ps[:, 0:D],
                        GT_sb[:, dt, et * P : (et + 1) * P],
                        w_sb[:, dt, 768:1024],
                        dt == 0,
                        dt == IT - 1,
                    )
                nc.vector.tensor_copy(M_sb[:, et, :], ps[:, 0:D])

            # --- outT[do, s] = M^T q^T + bo (per-partition bias, fp16) ---
            for dot in range(IT):
                for blk in range(SQ // 512):
                    ss = slice(blk * 512, (blk + 1) * 512)
                    ps = psQ.tile([P, 512], FP, tag="psQ")
                    for et in range(IT):
                        mm(
                            ps[:, :],
                            M_sb[:, et, dot * P : (dot + 1) * P],
                            qT_sb[:, et, ss],
                            et == 0,
                            et == IT - 1,
                        )
                    nc.vector.tensor_scalar_add(
                        outT_sb[:, dot, ss], ps[:, :], bias_sb[:, 2 + dot : 3 + dot]
                    )
                    if blk % 2 == 1:
                        off = dot * SQ + (blk - 1) * 512
                        src = outT_sb[:, dot, (blk - 1) * 512 : (blk + 1) * 512]
                        last = dot == IT - 1 and blk == SQ // 512 - 1
                        if last:
                            nc.sync.dma_start(
                                outp_d.ap()[0:64, off : off + 1024], src[0:64, :]
                            )
                            nc.gpsimd.dma_start(
                                outp_d.ap()[64:P, off : off + 1024], src[64:P, :]
                            )
                        else:
                            nc.sync.dma_start(outp_d.ap()[:, off : off + 1024], src)

    import concourse.hw_specs as hw_specs

    orig = bacc.get_activation_tables
    bacc.get_activation_tables = _patched_act_tables(hw_specs.get_activation_tables)
    try:
        nc.compile()
    finally:
        bacc.get_activation_tables = orig
    return nc


def _get_nc():
    nc = _CACHE.get("nc")
    if nc is None:
        nc = _build_nc()
        _CACHE["nc"] = nc
    return nc


def make_in_maps(x, Wq, bq, Wk, bk, Wv, bv, Wo, bo):
    B = x.shape[0]
    f16 = np.float16
    xf = np.asarray(x, dtype=np.float32).reshape(B, S, D)
    xfT = np.ascontiguousarray(xf.transpose(0, 2, 1).astype(f16))  # [B, 256, 4096]

    def pack_it(wT):  # [256, C] -> [128, 2*C] with it-blocks side by side
        C = wT.shape[1]
        return np.ascontiguousarray(
            wT.reshape(IT, P, C).transpose(1, 0, 2).reshape(P, IT * C)
        )

    wkvp = pack_it(np.hstack([np.asarray(Wk, f16).T, np.asarray(Wv, f16).T]))
    wqop = pack_it(np.hstack([np.asarray(Wq, f16).T, np.asarray(Wo, f16).T]))
    biasp = np.stack(
        [
            np.asarray(bq, np.float32)[0:P],
            np.asarray(bq, np.float32)[P:D],
            np.asarray(bo, np.float32)[0:P],
            np.asarray(bo, np.float32)[P:D],
        ],
        axis=1,
    )
    shared = {
        "wkvp": wkvp,
        "wqop": wqop,
        "biasp": np.ascontiguousarray(biasp),
        "bkv": np.concatenate(
            [np.asarray(bk, np.float32), np.asarray(bv, np.float32)]
        ).reshape(1, 2 * D),
    }
    in_maps = []
    for c in range(N_CORES):
        b, h = divmod(c, 2)
        xT = xfT[b]
        if h == 1:
            xT = np.concatenate([xT[:, SQ:], xT[:, :SQ]], axis=1)
        # pieces [it*4+cc] = [128, 1024]
        xpc = np.ascontiguousarray(
            xT.reshape(IT, P, 4, 1024).transpose(0, 2, 1, 3).reshape(8, P, 1024)
        )
        in_maps.append({"xp": xpc, **shared})
    return in_maps


def assemble_out(results, x_shape):
    B, S_, H, W = x_shape
    out = np.empty((B, S_, D), np.float32)
    for c in range(N_CORES):
        b, h = divmod(c, 2)
        outp = results[c]["outp"]  # [128, 2*SQ] fp16: [p, dot*SQ + s]
        v = outp.reshape(P, IT, SQ).astype(np.float32)
        out[b, h * SQ : (h + 1) * SQ] = v.transpose(2, 1, 0).reshape(SQ, D)
    return out.reshape(B, S_, H, W)


def kernel(x, Wq, bq, Wk, bk, Wv, bv, Wo, bo, _trace=False):
    from concourse.bass_utils import run_bass_kernel_spmd

    nc = _get_nc()
    in_maps = make_in_maps(x, Wq, bq, Wk, bk, Wv, bv, Wo, bo)
    res = run_bass_kernel_spmd(nc, in_maps, list(range(N_CORES)), trace=_trace)
    out = assemble_out(res.results, x.shape)
    if _trace:
        _CACHE["last_result"] = res
    return out
